# revision 28
# baseline (speedup 1.0000x reference)
"""Trainium2 Bass kernel for the non-local attention block (nn_CPP_80676665688885).

Sharding: pure data-parallel over batch - 1 sample per NeuronCore (B=8, 8 cores).
BatchNorm batch-statistics are combined with a tiny (2 KB) AllReduce.

Precision scheme (validated vs reference in numpy, rel-err ~8e-3 < 2e-2):
  - All matmuls single-pass: stationary (weight) operands bf16 (fast weight
    load), moving operands float32r (1 cyc/col when free >= 256) or bf16.
  - exp(fT) emitted as bf16 (correlated perturbation cancels in softmax).
  - Softmax denominators computed by an all-ones (128,128) bf16 matmul so the
    PSUM result is already broadcast across partitions; 1/s via
    reciprocal_approx_fast (~18 bits), y_n = yps * r elementwise.
  - Biases bp, bg, bw cancel mathematically (bp: per-n constant in softmax
    logits; bg: additive constant absorbed by BatchNorm mean; bw: same).
    Only bt survives and is folded into the theta PSUM->SBUF bias-add.

Per-core algorithm (sample x: (C=256, N=4096), N = 64x64 spatial):
  theta = Wt@x + bt  (f32); phi,g = maxpool2(conv) pooled straight out of
  PSUM (DVE for phi, Pool engine for g), stored bf16
  gT    = transpose(g_pool) via PE (bf16)
  per n-tile (512): fT = phi_mc^T @ theta; ef = exp(fT) bf16
     y += gT_mc^T @ ef ; s += ones^T @ ef (s lands broadcast on 128 parts)
     y_n = y * approx(1/s); wy_ch = Ww_ch @ y_n -> bf16 + S1/S2 accumulation
  stats AllReduce (warmed by dummy AllReduces during compute)
  z = scale*wy + x (bf16), out = max_n z + shift
"""

import numpy as np
from contextlib import ExitStack

import concourse.bass as bass
import concourse.bacc as bacc
import concourse.tile as tile
from concourse import mybir
from concourse.bass_utils import run_bass_kernel_spmd

F32 = mybir.dt.float32
F32R = mybir.dt.float32r
BF16 = mybir.dt.bfloat16
AF = mybir.ActivationFunctionType
ALU = mybir.AluOpType
AX = mybir.AxisListType

B = 8
C = 256
CI = 128
N = 4096          # 64*64
M = 1024          # 32*32 after 2x2 maxpool
NT = 512          # n-tile (PSUM bank width in fp32)
NTILES = N // NT  # 8
MCH = M // 128    # 8 m-chunks
CCH = C // 128    # 2 channel chunks
EPS = 1e-5
INV_CNT = 1.0 / (B * N)

_CACHE = {}
_LAST = {}


def _build():
    import ml_dtypes

    nc = bacc.Bacc("TRN2", num_devices=B)

    x_d = nc.declare_dram_parameter("x", [C, N], F32R, False)
    wT_d = {nm: nc.declare_dram_parameter(f"W{nm}T", [C, CI], F32R, False)
            for nm in ("t", "p", "g")}
    wwT_d = nc.declare_dram_parameter("WwT", [CI, C], BF16, False)
    bt_d = nc.declare_dram_parameter("bt", [CI, 1], F32, False)
    gamma_d = nc.declare_dram_parameter("gamma", [128, CCH], F32, False)
    beta_d = nc.declare_dram_parameter("beta", [128, CCH], F32, False)
    out_d = nc.declare_dram_parameter("out", [CCH, 128], F32, True)

    ident_bf_d = nc.inline_tensor(
        np.eye(128).astype(ml_dtypes.bfloat16), name="identbf")

    warm_in = [nc.dram_tensor(f"warm_in{i}", [128, 2 * CCH], F32)
               for i in range(3)]
    warm_out = [nc.dram_tensor(f"warm_out{i}", [128, 2 * CCH], F32,
                           addr_space="Shared") for i in range(3)]
    stats_in = nc.dram_tensor("stats_in", [128, 2 * CCH], F32)
    stats_out = nc.dram_tensor("stats_out", [128, 2 * CCH], F32,
                               addr_space="Shared")

    with ExitStack() as ctx:
        tc = ctx.enter_context(tile.TileContext(nc))
        consts = ctx.enter_context(tc.tile_pool(name="consts", bufs=1))
        persist = ctx.enter_context(tc.tile_pool(name="persist", bufs=1))
        mids = ctx.enter_context(tc.tile_pool(name="mids", bufs=2))
        efp = ctx.enter_context(tc.tile_pool(name="efp", bufs=4))
        nrm = ctx.enter_context(tc.tile_pool(name="nrm", bufs=2))
        small = ctx.enter_context(tc.tile_pool(name="small", bufs=4))
        ps_cv = ctx.enter_context(tc.tile_pool(name="ps_cv", bufs=2, space="PSUM"))
        ps_ft = ctx.enter_context(tc.tile_pool(name="ps_ft", bufs=2, space="PSUM"))
        ps_y = ctx.enter_context(tc.tile_pool(name="ps_y", bufs=2, space="PSUM"))
        ps_s = ctx.enter_context(tc.tile_pool(name="ps_s", bufs=2, space="PSUM"))

        # ---- projection weights first on the gpsimd queue ----
        w_sb = {}
        for nm in ("t", "p", "g"):
            w_sb[nm] = consts.tile([128, CCH, CI], F32R, name=f"w_{nm}")
            for ch in range(CCH):
                nc.gpsimd.dma_start(out=w_sb[nm][:, ch, :],
                                    in_=wT_d[nm][ch * 128:(ch + 1) * 128, :])

        # warmup collective (still ~90us before the real one)
        warm_sb = small.tile([128, 2 * CCH], F32, tag="warm")
        nc.vector.memset(warm_sb, 1.0)
        nc.gpsimd.dma_start(out=warm_in[0][:, :], in_=warm_sb)
        nc.gpsimd.collective_compute(
            "AllReduce", ALU.add, replica_groups=[list(range(B))],
            ins=[warm_in[0][:, :]], outs=[warm_out[0][:, :]])

        ww_sb = consts.tile([128, CCH, 128], BF16)
        for ch in range(CCH):
            nc.gpsimd.dma_start(out=ww_sb[:, ch, :],
                                in_=wwT_d[:, ch * 128:(ch + 1) * 128])
        # small constants on the gpsimd queue
        ident_bf = consts.tile([128, 128], BF16)
        nc.gpsimd.dma_start(out=ident_bf, in_=ident_bf_d[:, :])
        ones_bf = consts.tile([128, 128], BF16)
        nc.vector.memset(ones_bf, 1.0)
        bt_sb = consts.tile([128, 1], F32)
        nc.gpsimd.dma_start(out=bt_sb, in_=bt_d[:, :])
        gamma_sb = consts.tile([128, CCH], F32)
        beta_sb = consts.tile([128, CCH], F32)
        nc.gpsimd.dma_start(out=gamma_sb, in_=gamma_d[:, :])
        nc.gpsimd.dma_start(out=beta_sb, in_=beta_d[:, :])
        eps_sb = consts.tile([128, 1], F32)
        nc.vector.memset(eps_sb, EPS)

        # ---- x: two half-chunk DMAs per channel chunk ----
        x_sb = [persist.tile([128, N], F32R, tag=f"x{ch}", name=f"x{ch}")
                for ch in range(CCH)]
        for a, b in ((0, 512), (512, 1024), (1024, 2048), (2048, 3072),
                     (3072, 4096)):
            nc.sync.dma_start(out=x_sb[0][:, a:b], in_=x_d[0:128, a:b])
            nc.scalar.dma_start(out=x_sb[1][:, a:b], in_=x_d[128:256, a:b])

        # ---- stage B: projections + fused pooling ----
        theta = persist.tile([128, N], BF16, tag="theta")
        phi_pool = persist.tile([128, M], BF16, tag="phip")
        g_pool = persist.tile([128, M], BF16, tag="gp")

        def conv2(ps, nm, sl):
            nc.tensor.matmul(ps, lhsT=w_sb[nm][:, 0, :],
                             rhs=x_sb[0][:, sl],
                             start=True, stop=False)
            nc.tensor.matmul(ps, lhsT=w_sb[nm][:, 1, :],
                             rhs=x_sb[1][:, sl],
                             start=False, stop=True)

        for it in range(NTILES):
            sl = slice(it * NT, (it + 1) * NT)
            msl = slice(it * 128, (it + 1) * 128)
            ps = ps_cv.tile([128, NT], F32, tag="cv")
            conv2(ps, "t", sl)
            nc.scalar.activation(out=theta[:, sl], in_=ps, func=AF.Identity,
                                 bias=bt_sb, scale=1.0)
            # 2x2 maxpool fused out of PSUM: one XY-reduce over the
            # (row-pair, col-pair) innermost axes per projection
            for nm, dst, ptag, ppool in (("p", phi_pool, "ft", ps_ft),
                                         ("g", g_pool, "y", ps_y)):
                ps2 = ppool.tile([128, NT], F32, tag=ptag, name=f"cv_{nm}")
                conv2(ps2, nm, sl)
                pr = ps2.rearrange("p (hp s wp t) -> p hp wp s t",
                                   hp=4, s=2, wp=32, t=2)
                nc.vector.tensor_reduce(
                    out=dst[:, msl].rearrange("p (hp wp) -> p hp wp", hp=4),
                    in_=pr, axis=AX.XY, op=ALU.max)

        # ---- transpose g_pool -> gT (bf16, via PE; one PSUM bank) ----
        gT = persist.tile([128, MCH, CI], BF16, tag="gT")
        tp = ps_cv.tile([128, MCH, 128], BF16, tag="cv")
        for mc in range(MCH):
            nc.tensor.transpose(tp[:, mc, :],
                                g_pool[:, mc * 128:(mc + 1) * 128], ident_bf)
        nc.vector.tensor_copy(out=gT[:, :, :], in_=tp[:, :, :])

        # ---- attention + normalization + W-conv, per n-tile ----
        wy = [persist.tile([128, N], BF16, tag=f"wy{ch}", name=f"wy{ch}")
              for ch in range(CCH)]
        x_bf = [persist.tile([128, N], BF16, tag=f"xb{ch}", name=f"xb{ch}")
                for ch in range(CCH)]
        s1p = persist.tile([128, CCH, NTILES], F32, tag="s1p")
        s2p = persist.tile([128, CCH, NTILES], F32, tag="s2p")
        sq_trash = persist.tile([128, NT], BF16, tag="sqt")

        def norm_wconv(pit, psl, pyps, psps):
            rb = nrm.tile([128, NT], F32, tag="rb")
            nc.vector.reciprocal_approx_fast(out=rb, in_=psps)
            y_n = nrm.tile([128, NT], BF16, tag="yn")
            nc.vector.tensor_tensor(out=y_n, in0=pyps, in1=rb, op=ALU.mult)
            for ch in range(CCH):
                wps = ps_cv.tile([128, NT], F32, tag="cv")
                nc.tensor.matmul(wps, lhsT=ww_sb[:, ch, :],
                                 rhs=y_n, start=True, stop=True)
                if ch == 0:
                    # offload one PSUM->SBUF copy (+S1 accum) to ACT
                    nc.scalar.activation(
                        out=wy[0][:, psl], in_=wps, func=AF.Copy,
                        accum_out=s1p[:, 0, pit:pit + 1])
                else:
                    nc.vector.tensor_scalar(
                        out=wy[1][:, psl], in0=wps, scalar1=0.0,
                        scalar2=None, op0=ALU.add, op1=ALU.add,
                        accum_out=s1p[:, 1, pit:pit + 1])
                nc.vector.scalar_tensor_tensor(
                    out=sq_trash, in0=wy[ch][:, psl], scalar=1.0,
                    in1=wy[ch][:, psl], op0=ALU.mult, op1=ALU.mult,
                    accum_out=s2p[:, ch, pit:pit + 1])
            for ch in range(CCH):
                nc.gpsimd.tensor_copy(out=x_bf[ch][:, psl],
                                      in_=x_sb[ch][:, psl])

        pend = None
        for it in range(NTILES):
            sl = slice(it * NT, (it + 1) * NT)
            yps = ps_y.tile([128, NT], F32, tag="y")
            sps = ps_s.tile([128, NT], F32, tag="s")
            prev_ef = None
            for mc in range(MCH):
                fps = ps_ft.tile([128, NT], F32, tag="ft")
                nc.tensor.matmul(fps,
                                 lhsT=phi_pool[:, mc * 128:(mc + 1) * 128],
                                 rhs=theta[:, sl],
                                 start=True, stop=True)
                ef = efp.tile([128, NT], BF16, tag="ef")
                nc.scalar.activation(out=ef, in_=fps, func=AF.Exp)
                nc.tensor.matmul(yps, lhsT=gT[:, mc, :], rhs=ef,
                                 start=(mc == 0), stop=(mc == MCH - 1))
                if mc % 2 == 0:
                    prev_ef = ef
                else:
                    # one bf16 pair-add level halves the ones-matmul count
                    pe_t = nrm.tile([128, NT], BF16, tag="pe", name="pe_t")
                    nc.vector.tensor_tensor(out=pe_t, in0=prev_ef, in1=ef,
                                            op=ALU.add)
                    nc.tensor.matmul(sps, lhsT=ones_bf, rhs=pe_t,
                                     start=(mc == 1), stop=(mc == MCH - 1))
                if mc == 1 and pend is not None:
                    # deferred normalize+wconv of the previous tile: the PE
                    # fills the y_n dependency wait with this tile's fT work
                    norm_wconv(*pend)
                    pend = None
            pend = (it, sl, yps, sps)
            if it in (3, 6):
                i = 1 if it == 3 else 2
                nc.gpsimd.dma_start(out=warm_in[i][:, :], in_=warm_sb)
                nc.gpsimd.collective_compute(
                    "AllReduce", ALU.add, replica_groups=[list(range(B))],
                    ins=[warm_in[i][:, :]], outs=[warm_out[i][:, :]])
        norm_wconv(*pend)

        # ---- stats, AllReduce, finalize ----
        stats_sb = small.tile([128, 2 * CCH], F32, tag="stats")
        for ch in range(CCH):
            nc.vector.tensor_reduce(out=stats_sb[:, 2 * ch:2 * ch + 1],
                                    in_=s1p[:, ch, :], axis=AX.X, op=ALU.add)
            nc.vector.tensor_reduce(out=stats_sb[:, 2 * ch + 1:2 * ch + 2],
                                    in_=s2p[:, ch, :], axis=AX.X, op=ALU.add)
        nc.gpsimd.dma_start(out=stats_in[:, :], in_=stats_sb)
        nc.gpsimd.collective_compute(
            "AllReduce", ALU.add, replica_groups=[list(range(B))],
            ins=[stats_in[:, :]], outs=[stats_out[:, :]])
        # preload the Sqrt activation table while the AllReduce is in flight
        sqrt_dummy = small.tile([128, 1], F32, tag="sqd")
        nc.scalar.activation(out=sqrt_dummy, in_=eps_sb, func=AF.Sqrt,
                             bias=eps_sb, scale=1.0)
        stats_g = small.tile([128, 2 * CCH], F32, tag="statsg")
        nc.sync.dma_start(out=stats_g, in_=stats_out[:, :])

        zt = persist.tile([128, N], BF16, tag="zt")
        out_sb = small.tile([128, CCH], F32, tag="outsb")
        mean2 = small.tile([128, CCH], F32, tag="fin")
        var2 = small.tile([128, CCH], F32, tag="fin")
        sd2 = small.tile([128, CCH], F32, tag="fin")
        inv2 = small.tile([128, CCH], F32, tag="fin")
        scale2 = small.tile([128, CCH], F32, tag="fin")
        nshift2 = small.tile([128, CCH], F32, tag="fin")
        nc.vector.tensor_scalar_mul(out=mean2, in0=stats_g[:, 0:2 * CCH:2],
                                    scalar1=INV_CNT)
        nc.vector.tensor_scalar_mul(out=var2, in0=stats_g[:, 1:2 * CCH:2],
                                    scalar1=INV_CNT)
        m22 = small.tile([128, CCH], F32, tag="fin")
        nc.vector.tensor_tensor(out=m22, in0=mean2, in1=mean2, op=ALU.mult)
        nc.vector.tensor_tensor(out=var2, in0=var2, in1=m22, op=ALU.subtract)
        nc.scalar.activation(out=sd2, in_=var2, func=AF.Sqrt, bias=eps_sb,
                             scale=1.0)
        nc.vector.reciprocal(out=inv2, in_=sd2)
        nc.vector.tensor_tensor(out=scale2, in0=inv2, in1=gamma_sb,
                                op=ALU.mult)
        nc.vector.scalar_tensor_tensor(out=nshift2, in0=mean2, scalar=1.0,
                                       in1=beta_sb, op0=ALU.mult,
                                       op1=ALU.subtract)
        nc.vector.tensor_tensor(out=nshift2, in0=scale2, in1=mean2,
                                op=ALU.mult)
        nc.vector.tensor_tensor(out=nshift2, in0=nshift2, in1=beta_sb,
                                op=ALU.subtract)
        for ch in range(CCH):
            # u = scale*wy on ACT (per-partition scale is native there),
            # then all-bf16 DVE add + pairwise-max tree (2x mode)
            for h in range(2):
                hs = slice(h * (N // 2), (h + 1) * (N // 2))
                ut = persist.tile([128, N // 2], BF16, tag=f"ut{h}",
                                  name=f"ut{h}")
                nc.scalar.activation(out=ut, in_=wy[ch][:, hs], func=AF.Copy,
                                     bias=0.0, scale=scale2[:, ch:ch + 1])
                nc.vector.tensor_tensor(out=zt[:, hs], in0=ut,
                                        in1=x_bf[ch][:, hs], op=ALU.add)
            w = N // 2
            while w >= 256:
                nc.vector.tensor_tensor(out=zt[:, :w], in0=zt[:, :w],
                                        in1=zt[:, w:2 * w], op=ALU.max)
                w //= 2
            mx = small.tile([128, 1], F32, tag="fin")
            nc.vector.tensor_reduce(out=mx, in_=zt[:, :256], axis=AX.X,
                                    op=ALU.max)
            nc.vector.tensor_tensor(out=out_sb[:, ch:ch + 1], in0=mx,
                                    in1=nshift2[:, ch:ch + 1],
                                    op=ALU.subtract)
        for ch in range(CCH):
            nc.sync.dma_start(
                out=out_d[ch, :].rearrange("(p one) -> p one", one=1),
                in_=out_sb[:, ch:ch + 1])

    nc.compile()
    return nc


def kernel(**inputs):
    x = np.ascontiguousarray(inputs["x"], dtype=np.float32)      # (8, 256, 64, 64)
    Wt = np.asarray(inputs["Wt"], dtype=np.float32)
    bt = np.asarray(inputs["bt"], dtype=np.float32)
    Wp = np.asarray(inputs["Wp"], dtype=np.float32)
    Wg = np.asarray(inputs["Wg"], dtype=np.float32)
    Ww = np.asarray(inputs["Ww"], dtype=np.float32)
    gamma = np.asarray(inputs["gamma"], dtype=np.float32)
    beta = np.asarray(inputs["beta"], dtype=np.float32)

    if "nc" not in _CACHE:
        _CACHE["nc"] = _build()
    nc = _CACHE["nc"]

    import ml_dtypes
    bf = ml_dtypes.bfloat16

    shared = {
        "WtT": np.ascontiguousarray(Wt.T),
        "WpT": np.ascontiguousarray(Wp.T),
        "WgT": np.ascontiguousarray(Wg.T),
        "WwT": np.ascontiguousarray(Ww.T.astype(bf)),
        "bt": np.ascontiguousarray(bt.reshape(CI, 1)),
        "gamma": np.ascontiguousarray(gamma.reshape(CCH, 128).T),
        "beta": np.ascontiguousarray(beta.reshape(CCH, 128).T),
    }
    in_maps = [dict(shared, x=np.ascontiguousarray(x[b].reshape(C, N)))
               for b in range(B)]
    import os
    trace = bool(int(os.environ.get("KERNEL_TRACE", "0")))
    res = run_bass_kernel_spmd(nc, in_maps, core_ids=list(range(B)), trace=trace)
    _LAST["res"] = res
    out = np.stack([np.asarray(res.results[b]["out"]).astype(np.float32).reshape(C)
                    for b in range(B)])
    return out.reshape(B, C, 1, 1).astype(np.float32)


if __name__ == "__main__":
    pass


# revision 29
# speedup vs baseline: 1.2102x; 1.2102x over previous
"""Trainium2 Bass kernel for the non-local attention block (nn_CPP_80676665688885).

Sharding: pure data-parallel over batch - 1 sample per NeuronCore (B=8, 8 cores).
BatchNorm batch-statistics are combined with a tiny (2 KB) AllReduce.

Precision scheme (validated vs reference in numpy, rel-err ~8e-3 < 2e-2):
  - All matmuls single-pass: stationary (weight) operands bf16 (fast weight
    load), moving operands float32r (1 cyc/col when free >= 256) or bf16.
  - exp(fT) emitted as bf16 (correlated perturbation cancels in softmax).
  - Softmax denominators computed by an all-ones (128,128) bf16 matmul so the
    PSUM result is already broadcast across partitions; 1/s via
    reciprocal_approx_fast (~18 bits), y_n = yps * r elementwise.
  - Biases bp, bg, bw cancel mathematically (bp: per-n constant in softmax
    logits; bg: additive constant absorbed by BatchNorm mean; bw: same).
    Only bt survives and is folded into the theta PSUM->SBUF bias-add.

Per-core algorithm (sample x: (C=256, N=4096), N = 64x64 spatial):
  theta = Wt@x + bt  (f32); phi,g = maxpool2(conv) pooled straight out of
  PSUM (DVE for phi, Pool engine for g), stored bf16
  gT    = transpose(g_pool) via PE (bf16)
  per n-tile (512): fT = phi_mc^T @ theta; ef = exp(fT) bf16
     y += gT_mc^T @ ef ; s += ones^T @ ef (s lands broadcast on 128 parts)
     y_n = y * approx(1/s); wy_ch = Ww_ch @ y_n -> bf16 + S1/S2 accumulation
  stats AllReduce (warmed by dummy AllReduces during compute)
  z = scale*wy + x (bf16), out = max_n z + shift
"""

import numpy as np
from contextlib import ExitStack

import concourse.bass as bass
import concourse.bacc as bacc
import concourse.tile as tile
from concourse import mybir
from concourse.bass_utils import run_bass_kernel_spmd

F32 = mybir.dt.float32
F32R = mybir.dt.float32r
BF16 = mybir.dt.bfloat16
AF = mybir.ActivationFunctionType
ALU = mybir.AluOpType
AX = mybir.AxisListType

B = 8
C = 256
CI = 128
N = 4096          # 64*64
M = 1024          # 32*32 after 2x2 maxpool
NT = 512          # n-tile (PSUM bank width in fp32)
NTILES = N // NT  # 8
MCH = M // 128    # 8 m-chunks
CCH = C // 128    # 2 channel chunks
EPS = 1e-5
INV_CNT = 1.0 / (B * N)

_CACHE = {}
_LAST = {}


def _build():
    import ml_dtypes

    nc = bacc.Bacc("TRN2", num_devices=B)

    x_d = nc.declare_dram_parameter("x", [C, N], F32R, False)
    wT_d = {nm: nc.declare_dram_parameter(f"W{nm}T", [C, CI], F32R, False)
            for nm in ("t", "p", "g")}
    wwT_d = nc.declare_dram_parameter("WwT", [CI, C], BF16, False)
    bt_d = nc.declare_dram_parameter("bt", [CI, 1], F32, False)
    gamma_d = nc.declare_dram_parameter("gamma", [128, CCH], F32, False)
    beta_d = nc.declare_dram_parameter("beta", [128, CCH], F32, False)
    out_d = nc.declare_dram_parameter("out", [CCH, 128], F32, True)

    ident_bf_d = nc.inline_tensor(
        np.eye(128).astype(ml_dtypes.bfloat16), name="identbf")

    warm_in = [nc.dram_tensor(f"warm_in{i}", [128, 2 * CCH], F32)
               for i in range(3)]
    warm_out = [nc.dram_tensor(f"warm_out{i}", [128, 2 * CCH], F32,
                           addr_space="Shared") for i in range(3)]
    stats_in = nc.dram_tensor("stats_in", [128, 2 * CCH], F32)
    stats_out = nc.dram_tensor("stats_out", [128, 2 * CCH], F32,
                               addr_space="Shared")

    with ExitStack() as ctx:
        tc = ctx.enter_context(tile.TileContext(nc))
        consts = ctx.enter_context(tc.tile_pool(name="consts", bufs=1))
        persist = ctx.enter_context(tc.tile_pool(name="persist", bufs=1))
        mids = ctx.enter_context(tc.tile_pool(name="mids", bufs=2))
        efp = ctx.enter_context(tc.tile_pool(name="efp", bufs=4))
        nrm = ctx.enter_context(tc.tile_pool(name="nrm", bufs=2))
        small = ctx.enter_context(tc.tile_pool(name="small", bufs=4))
        ps_cv = ctx.enter_context(tc.tile_pool(name="ps_cv", bufs=2, space="PSUM"))
        ps_ft = ctx.enter_context(tc.tile_pool(name="ps_ft", bufs=2, space="PSUM"))
        ps_y = ctx.enter_context(tc.tile_pool(name="ps_y", bufs=2, space="PSUM"))
        ps_s = ctx.enter_context(tc.tile_pool(name="ps_s", bufs=2, space="PSUM"))

        # ---- projection weights first on the gpsimd queue ----
        w_sb = {}
        for nm in ("t", "p", "g"):
            w_sb[nm] = consts.tile([128, CCH, CI], F32R, name=f"w_{nm}")
            for ch in range(CCH):
                nc.gpsimd.dma_start(out=w_sb[nm][:, ch, :],
                                    in_=wT_d[nm][ch * 128:(ch + 1) * 128, :])

        # warmup collective (still ~90us before the real one)
        warm_sb = small.tile([128, 2 * CCH], F32, tag="warm")
        nc.vector.memset(warm_sb, 1.0)
        nc.gpsimd.dma_start(out=warm_in[0][:, :], in_=warm_sb)
        nc.gpsimd.collective_compute(
            "AllReduce", ALU.add, replica_groups=[list(range(B))],
            ins=[warm_in[0][:, :]], outs=[warm_out[0][:, :]])

        ww_sb = consts.tile([128, CCH, 128], BF16)
        for ch in range(CCH):
            nc.gpsimd.dma_start(out=ww_sb[:, ch, :],
                                in_=wwT_d[:, ch * 128:(ch + 1) * 128])
        # small constants on the gpsimd queue
        ident_bf = consts.tile([128, 128], BF16)
        nc.gpsimd.dma_start(out=ident_bf, in_=ident_bf_d[:, :])
        ones_bf = consts.tile([128, 128], BF16)
        nc.vector.memset(ones_bf, 1.0)
        bt_sb = consts.tile([128, 1], F32)
        nc.gpsimd.dma_start(out=bt_sb, in_=bt_d[:, :])
        gamma_sb = consts.tile([128, CCH], F32)
        beta_sb = consts.tile([128, CCH], F32)
        nc.gpsimd.dma_start(out=gamma_sb, in_=gamma_d[:, :])
        nc.gpsimd.dma_start(out=beta_sb, in_=beta_d[:, :])
        eps_sb = consts.tile([128, 1], F32)
        nc.vector.memset(eps_sb, EPS)

        # ---- x: two half-chunk DMAs per channel chunk ----
        x_sb = [persist.tile([128, N], F32R, tag=f"x{ch}", name=f"x{ch}")
                for ch in range(CCH)]
        for a, b in ((0, 512), (512, 1024), (1024, 2048), (2048, 3072),
                     (3072, 4096)):
            nc.sync.dma_start(out=x_sb[0][:, a:b], in_=x_d[0:128, a:b])
            nc.scalar.dma_start(out=x_sb[1][:, a:b], in_=x_d[128:256, a:b])

        # ---- stage B: projections + fused pooling ----
        theta = persist.tile([128, N], BF16, tag="theta")
        phi_pool = persist.tile([128, M], BF16, tag="phip")
        g_pool = persist.tile([128, M], BF16, tag="gp")

        def conv2(ps, nm, sl):
            nc.tensor.matmul(ps, lhsT=w_sb[nm][:, 0, :],
                             rhs=x_sb[0][:, sl],
                             start=True, stop=False)
            nc.tensor.matmul(ps, lhsT=w_sb[nm][:, 1, :],
                             rhs=x_sb[1][:, sl],
                             start=False, stop=True)

        for it in range(NTILES):
            sl = slice(it * NT, (it + 1) * NT)
            msl = slice(it * 128, (it + 1) * 128)
            ps = ps_cv.tile([128, NT], F32, tag="cv")
            conv2(ps, "t", sl)
            nc.scalar.activation(out=theta[:, sl], in_=ps, func=AF.Identity,
                                 bias=bt_sb, scale=1.0)
            # 2x2 maxpool fused out of PSUM: one XY-reduce over the
            # (row-pair, col-pair) innermost axes per projection
            for nm, dst, ptag, ppool in (("p", phi_pool, "ft", ps_ft),
                                         ("g", g_pool, "y", ps_y)):
                ps2 = ppool.tile([128, NT], F32, tag=ptag, name=f"cv_{nm}")
                conv2(ps2, nm, sl)
                pr = ps2.rearrange("p (hp s wp t) -> p hp wp s t",
                                   hp=4, s=2, wp=32, t=2)
                nc.vector.tensor_reduce(
                    out=dst[:, msl].rearrange("p (hp wp) -> p hp wp", hp=4),
                    in_=pr, axis=AX.XY, op=ALU.max)

        # ---- transpose g_pool -> gT (bf16, via PE; one PSUM bank) ----
        gT = persist.tile([128, MCH, CI], BF16, tag="gT")
        tp = ps_cv.tile([128, MCH, 128], BF16, tag="cv")
        for mc in range(MCH):
            nc.tensor.transpose(tp[:, mc, :],
                                g_pool[:, mc * 128:(mc + 1) * 128], ident_bf)
        nc.vector.tensor_copy(out=gT[:, :, :], in_=tp[:, :, :])

        # ---- attention + normalization + W-conv, per n-tile ----
        wy = [persist.tile([128, N], BF16, tag=f"wy{ch}", name=f"wy{ch}")
              for ch in range(CCH)]
        x_bf = [persist.tile([128, N], BF16, tag=f"xb{ch}", name=f"xb{ch}")
                for ch in range(CCH)]
        s1p = persist.tile([128, CCH, NTILES], F32, tag="s1p")
        s2p = persist.tile([128, CCH, NTILES], F32, tag="s2p")
        sq_trash = persist.tile([128, NT], BF16, tag="sqt")

        def norm_wconv(pit, psl, pyps, psps):
            rb = nrm.tile([128, NT], F32, tag="rb")
            nc.vector.reciprocal_approx_fast(out=rb, in_=psps)
            y_n = nrm.tile([128, NT], BF16, tag="yn")
            nc.vector.tensor_tensor(out=y_n, in0=pyps, in1=rb, op=ALU.mult)
            for ch in range(CCH):
                wps = ps_cv.tile([128, NT], F32, tag="cv")
                nc.tensor.matmul(wps, lhsT=ww_sb[:, ch, :],
                                 rhs=y_n, start=True, stop=True)
                if ch == 0:
                    # offload one PSUM->SBUF copy (+S1 accum) to ACT
                    nc.scalar.activation(
                        out=wy[0][:, psl], in_=wps, func=AF.Copy,
                        accum_out=s1p[:, 0, pit:pit + 1])
                else:
                    nc.vector.tensor_scalar(
                        out=wy[1][:, psl], in0=wps, scalar1=0.0,
                        scalar2=None, op0=ALU.add, op1=ALU.add,
                        accum_out=s1p[:, 1, pit:pit + 1])
                nc.vector.scalar_tensor_tensor(
                    out=sq_trash, in0=wy[ch][:, psl], scalar=1.0,
                    in1=wy[ch][:, psl], op0=ALU.mult, op1=ALU.mult,
                    accum_out=s2p[:, ch, pit:pit + 1])
            for ch in range(CCH):
                nc.vector.tensor_copy(out=x_bf[ch][:, psl],
                                      in_=x_sb[ch][:, psl])

        pend = None
        for it in range(NTILES):
            sl = slice(it * NT, (it + 1) * NT)
            yps = ps_y.tile([128, NT], F32, tag="y")
            sps = ps_s.tile([128, NT], F32, tag="s")
            prev_ef = None
            for mc in range(MCH):
                fps = ps_ft.tile([128, NT], F32, tag="ft")
                nc.tensor.matmul(fps,
                                 lhsT=phi_pool[:, mc * 128:(mc + 1) * 128],
                                 rhs=theta[:, sl],
                                 start=True, stop=True)
                ef = efp.tile([128, NT], BF16, tag="ef")
                nc.scalar.activation(out=ef, in_=fps, func=AF.Exp)
                nc.tensor.matmul(yps, lhsT=gT[:, mc, :], rhs=ef,
                                 start=(mc == 0), stop=(mc == MCH - 1))
                if mc % 2 == 0:
                    prev_ef = ef
                else:
                    # one bf16 pair-add level halves the ones-matmul count
                    pe_t = nrm.tile([128, NT], BF16, tag="pe", name="pe_t")
                    nc.vector.tensor_tensor(out=pe_t, in0=prev_ef, in1=ef,
                                            op=ALU.add)
                    nc.tensor.matmul(sps, lhsT=ones_bf, rhs=pe_t,
                                     start=(mc == 1), stop=(mc == MCH - 1))
                if mc == 1 and pend is not None:
                    # deferred normalize+wconv of the previous tile: the PE
                    # fills the y_n dependency wait with this tile's fT work
                    norm_wconv(*pend)
                    pend = None
            pend = (it, sl, yps, sps)
            if it in (3, 6):
                i = 1 if it == 3 else 2
                nc.gpsimd.dma_start(out=warm_in[i][:, :], in_=warm_sb)
                nc.gpsimd.collective_compute(
                    "AllReduce", ALU.add, replica_groups=[list(range(B))],
                    ins=[warm_in[i][:, :]], outs=[warm_out[i][:, :]])
        norm_wconv(*pend)

        # ---- stats, AllReduce, finalize ----
        stats_sb = small.tile([128, 2 * CCH], F32, tag="stats")
        for ch in range(CCH):
            nc.vector.tensor_reduce(out=stats_sb[:, 2 * ch:2 * ch + 1],
                                    in_=s1p[:, ch, :], axis=AX.X, op=ALU.add)
            nc.vector.tensor_reduce(out=stats_sb[:, 2 * ch + 1:2 * ch + 2],
                                    in_=s2p[:, ch, :], axis=AX.X, op=ALU.add)
        nc.gpsimd.dma_start(out=stats_in[:, :], in_=stats_sb)
        nc.gpsimd.collective_compute(
            "AllReduce", ALU.add, replica_groups=[list(range(B))],
            ins=[stats_in[:, :]], outs=[stats_out[:, :]])
        # preload the Sqrt activation table while the AllReduce is in flight
        sqrt_dummy = small.tile([128, 1], F32, tag="sqd")
        nc.scalar.activation(out=sqrt_dummy, in_=eps_sb, func=AF.Sqrt,
                             bias=eps_sb, scale=1.0)
        stats_g = small.tile([128, 2 * CCH], F32, tag="statsg")
        nc.sync.dma_start(out=stats_g, in_=stats_out[:, :])

        zt = persist.tile([128, N], BF16, tag="zt")
        out_sb = small.tile([128, CCH], F32, tag="outsb")
        mean2 = small.tile([128, CCH], F32, tag="fin")
        var2 = small.tile([128, CCH], F32, tag="fin")
        sd2 = small.tile([128, CCH], F32, tag="fin")
        inv2 = small.tile([128, CCH], F32, tag="fin")
        scale2 = small.tile([128, CCH], F32, tag="fin")
        nshift2 = small.tile([128, CCH], F32, tag="fin")
        nc.vector.tensor_scalar_mul(out=mean2, in0=stats_g[:, 0:2 * CCH:2],
                                    scalar1=INV_CNT)
        nc.vector.tensor_scalar_mul(out=var2, in0=stats_g[:, 1:2 * CCH:2],
                                    scalar1=INV_CNT)
        m22 = small.tile([128, CCH], F32, tag="fin")
        nc.vector.tensor_tensor(out=m22, in0=mean2, in1=mean2, op=ALU.mult)
        nc.vector.tensor_tensor(out=var2, in0=var2, in1=m22, op=ALU.subtract)
        nc.scalar.activation(out=sd2, in_=var2, func=AF.Sqrt, bias=eps_sb,
                             scale=1.0)
        nc.vector.reciprocal(out=inv2, in_=sd2)
        nc.vector.tensor_tensor(out=scale2, in0=inv2, in1=gamma_sb,
                                op=ALU.mult)
        nc.vector.scalar_tensor_tensor(out=nshift2, in0=mean2, scalar=1.0,
                                       in1=beta_sb, op0=ALU.mult,
                                       op1=ALU.subtract)
        nc.vector.tensor_tensor(out=nshift2, in0=scale2, in1=mean2,
                                op=ALU.mult)
        nc.vector.tensor_tensor(out=nshift2, in0=nshift2, in1=beta_sb,
                                op=ALU.subtract)
        for ch in range(CCH):
            # u = scale*wy on ACT (per-partition scale is native there),
            # then all-bf16 DVE add + pairwise-max tree (2x mode)
            for h in range(2):
                hs = slice(h * (N // 2), (h + 1) * (N // 2))
                ut = persist.tile([128, N // 2], BF16, tag=f"ut{h}",
                                  name=f"ut{h}")
                nc.scalar.activation(out=ut, in_=wy[ch][:, hs], func=AF.Copy,
                                     bias=0.0, scale=scale2[:, ch:ch + 1])
                nc.vector.tensor_tensor(out=zt[:, hs], in0=ut,
                                        in1=x_bf[ch][:, hs], op=ALU.add)
            w = N // 2
            while w >= 256:
                nc.vector.tensor_tensor(out=zt[:, :w], in0=zt[:, :w],
                                        in1=zt[:, w:2 * w], op=ALU.max)
                w //= 2
            mx = small.tile([128, 1], F32, tag="fin")
            nc.vector.tensor_reduce(out=mx, in_=zt[:, :256], axis=AX.X,
                                    op=ALU.max)
            nc.vector.tensor_tensor(out=out_sb[:, ch:ch + 1], in0=mx,
                                    in1=nshift2[:, ch:ch + 1],
                                    op=ALU.subtract)
        for ch in range(CCH):
            nc.sync.dma_start(
                out=out_d[ch, :].rearrange("(p one) -> p one", one=1),
                in_=out_sb[:, ch:ch + 1])

    nc.compile()
    return nc


def kernel(**inputs):
    x = np.ascontiguousarray(inputs["x"], dtype=np.float32)      # (8, 256, 64, 64)
    Wt = np.asarray(inputs["Wt"], dtype=np.float32)
    bt = np.asarray(inputs["bt"], dtype=np.float32)
    Wp = np.asarray(inputs["Wp"], dtype=np.float32)
    Wg = np.asarray(inputs["Wg"], dtype=np.float32)
    Ww = np.asarray(inputs["Ww"], dtype=np.float32)
    gamma = np.asarray(inputs["gamma"], dtype=np.float32)
    beta = np.asarray(inputs["beta"], dtype=np.float32)

    if "nc" not in _CACHE:
        _CACHE["nc"] = _build()
    nc = _CACHE["nc"]

    import ml_dtypes
    bf = ml_dtypes.bfloat16

    shared = {
        "WtT": np.ascontiguousarray(Wt.T),
        "WpT": np.ascontiguousarray(Wp.T),
        "WgT": np.ascontiguousarray(Wg.T),
        "WwT": np.ascontiguousarray(Ww.T.astype(bf)),
        "bt": np.ascontiguousarray(bt.reshape(CI, 1)),
        "gamma": np.ascontiguousarray(gamma.reshape(CCH, 128).T),
        "beta": np.ascontiguousarray(beta.reshape(CCH, 128).T),
    }
    in_maps = [dict(shared, x=np.ascontiguousarray(x[b].reshape(C, N)))
               for b in range(B)]
    import os
    trace = bool(int(os.environ.get("KERNEL_TRACE", "0")))
    res = run_bass_kernel_spmd(nc, in_maps, core_ids=list(range(B)), trace=trace)
    _LAST["res"] = res
    out = np.stack([np.asarray(res.results[b]["out"]).astype(np.float32).reshape(C)
                    for b in range(B)])
    return out.reshape(B, C, 1, 1).astype(np.float32)


if __name__ == "__main__":
    pass
